# revision 41
# baseline (speedup 1.0000x reference)
"""Trainium2 Bass kernel for nn_ChainLoss (LF-MMI style chain loss).

Split by graph size:
  - The 32 per-utterance numerator graphs are tiny (200 states, 600 edges);
    their forward recursions run EXACTLY on the host (vectorized float64
    numpy with per-step renormalisation, ~0.2s) while the device handles the
    heavy shared denominator graph (4000 states, 120k edges, 500 steps,
    batch 32).
  - Denominator on device, in exp-domain with STATIC rescaling: the expected
    per-step growth (from data statistics) is folded into the edge weights
    (w' = w * e^-c) so alpha stays within f32 range for the whole recursion
    (measured drift ~ +8 nats); the host adds c*len_u back at the end.

Device layout: state table A[4096 rows x 32 utts] (f32, stored 64-wide for
256B gather alignment). The 8 cores shard states: core c owns rows
512c..512c+511 (global in-degree round-robin relabel) and all in-edges
targeting them, pre-sorted into a padded grid of 4 partition-tiles.

The per-instruction dispatch overhead dominates on this target, so the step
loop is built from as few instructions as possible:
  AllGather shards -> table T; A[src] rows gathered in ceil(NIDX/4096)
  dma_gathers (firmware cap); x rows gathered once per 8-step chunk from an
  int8 table (256B descriptors); one Exp activation (int8 in, dequant via
  activation scale); two tensor_tensor mults over the whole [128, KTOT, B]
  grid; 4 per-tile reduces; shard writeback. Per-utterance lengths are
  handled by capture-at-end: at the <=32 distinct utterance-ending steps,
  alpha*exp(final_lp) is accumulated (masked by a DMA-broadcast indicator
  row); no per-step freezing is needed since later alpha values for ended
  utterances are never read.

Input staging (the dominant cost over the axon tunnel) is minimized: the
196MB x table is shipped int8 (linear quant, scale 6/127) and row-sharded
across the 8 cores (6MB each), then AllGathered on-device; index tables are
shipped as one 16-partition block and replicated on device; the w grid is
shipped as one weight per edge slot.
"""
import numpy as np

NCORES = 8
B = 32
T = 500
D = 3072
S_DEN = 4000
S_NUM = 200
SHARD = 512
NROWS = SHARD * NCORES      # 4096
XCH = 8                     # time steps per X-gather descriptor/chunk
NCHUNK = -(-T // XCH)       # 63 (time padded to 504)
XQS = 6.0 / 127.0           # int8 x quantization scale (randn tail-safe)


# ---------------------------------------------------------------- host prep
def _preprocess(den_src, den_dst, den_pdf, den_logw):
    indeg = np.bincount(den_dst, minlength=S_DEN)
    rank_of_state = np.empty(S_DEN, np.int64)
    rank_of_state[np.argsort(-indeg, kind="stable")] = np.arange(S_DEN)
    core_of = rank_of_state % NCORES
    rowin = rank_of_state // NCORES
    rowof_den = core_of * SHARD + rowin

    E = len(den_dst)
    core_e = core_of[den_dst]
    ri_e = rowin[den_dst]
    grp = core_e * SHARD + ri_e
    order = np.argsort(grp, kind="stable")
    grp_s = grp[order]
    first = np.r_[True, grp_s[1:] != grp_s[:-1]]
    start_pos = np.where(first, np.arange(E), 0)
    k_within = np.arange(E) - np.maximum.accumulate(start_pos)
    e_src = rowof_den[den_src[order]]
    e_pdf = den_pdf[order]
    e_w = np.exp(den_logw[order]).astype(np.float32)
    tile_s = ri_e[order] // 128
    part_s = ri_e[order] % 128
    core_s = core_e[order]

    per_core = [dict(aidx=[None] * 4, xidx=[None] * 4, w=[None] * 4)
                for _ in range(NCORES)]
    Kmax = [0] * 4
    raw = {}
    for c in range(NCORES):
        for j in range(4):
            sel = (core_s == c) & (tile_s == j)
            K = int(k_within[sel].max()) + 1 if sel.any() else 1
            Kmax[j] = max(Kmax[j], K)
            raw[(c, j)] = sel


    for c in range(NCORES):
        for j in range(4):
            K = Kmax[j]
            sel = raw[(c, j)]
            ai = np.zeros((128, K), np.int32)
            xi = np.zeros((128, K), np.int32)
            wt = np.zeros((128, K), np.float32)
            p, k = part_s[sel], k_within[sel]
            ai[p, k] = e_src[sel]
            xi[p, k] = e_pdf[sel]
            wt[p, k] = e_w[sel]
            pc = per_core[c]
            pc["aidx"][j] = ai; pc["xidx"][j] = xi; pc["w"][j] = wt

    return per_core, Kmax, rowof_den


def _num_ll_host(x, x_lengths, num_src, num_dst, num_pdf, num_logw,
                 num_init, num_final, n_steps):
    """Exact numerator forward recursions, vectorized over utterances.
    x_lengths is sorted descending, so the active set is always a prefix."""
    steps_u = np.minimum(x_lengths, n_steps).astype(np.int64)
    w = np.exp(num_logw.astype(np.float64))            # [B, E]
    a = np.exp(num_init.astype(np.float64))            # [B, S]
    logs = np.zeros(B)
    ui = np.arange(B)[:, None]
    flat_dst = ui * S_NUM + num_dst                    # [B, E]
    for t in range(int(steps_u.max())):
        k = int((steps_u > t).sum())                   # active prefix
        xp = x[np.arange(k)[:, None], t, num_pdf[:k]].astype(np.float64)
        s = a[np.arange(k)[:, None], num_src[:k]] * w[:k] * np.exp(xp)
        anew = np.bincount(flat_dst[:k].ravel(), weights=s.ravel(),
                           minlength=k * S_NUM)[:k * S_NUM].reshape(k, S_NUM)
        g = anew.sum(axis=1)
        logs[:k] += np.log(g)
        a[:k] = anew / g[:, None]
    fin = (a * np.exp(num_final.astype(np.float64))).sum(axis=1)
    return np.log(fin) + logs


# ------------------------------------------------------------- bass program
def _build(Kmax, n_steps, ends, ablate=""):
    import concourse.bass as bass
    import concourse.tile as tile
    from concourse import bacc, mybir

    f32 = mybir.dt.float32
    f16 = mybir.dt.float16
    i8 = mybir.dt.int8
    KTOT = sum(Kmax)
    NIDX = 128 * KTOT
    offs = np.cumsum([0] + Kmax).tolist()
    NEND = max(len(ends), 1)
    end_row = {t: i for i, t in enumerate(ends)}
    GCAP = 4096                 # firmware cap on indices per dma_gather

    nc = bacc.Bacc("TRN2", target_bir_lowering=False, debug=False,
                   num_devices=NCORES)
    core_ids = list(range(NCORES))

    XSH = NCHUNK * D // NCORES
    xtsh = nc.dram_tensor("xtsh", [XSH, XCH * B], i8, kind="ExternalInput").ap()
    aidx_in = nc.dram_tensor("aidx", [16, NIDX // 16], mybir.dt.int16, kind="ExternalInput").ap()
    xidx_in = nc.dram_tensor("xidx", [16, NIDX // 16], mybir.dt.int16, kind="ExternalInput").ap()
    w_in = nc.dram_tensor("wsm", [128, KTOT], f16, kind="ExternalInput").ap()
    fshard_in = nc.dram_tensor("fshard", [128, 4 * B], f32, kind="ExternalInput").ap()
    init64_in = nc.dram_tensor("init64", [SHARD, 64], f32, kind="ExternalInput").ap()
    iend_in = nc.dram_tensor("iend", [NEND, 128], f32, kind="ExternalInput").ap()
    out_t = nc.dram_tensor("out", [1, B], f32, kind="ExternalOutput").ap()

    shard64 = nc.dram_tensor("shard64", [SHARD, 64], f32).ap()
    TT = [nc.dram_tensor(f"table{i}", [NROWS, 64], f32, addr_space="Shared").ap()
          for i in range(2)]
    xstage = nc.dram_tensor("xstage", [XSH, XCH * B], i8).ap()
    xfull = nc.dram_tensor("xfull", [NCHUNK * D, XCH * B], i8,
                           addr_space="Shared").ap()

    with tile.TileContext(nc) as tc:
        with tc.tile_pool(name="main", bufs=1) as pool, \
             tc.tile_pool(name="psum", bufs=1, space="PSUM") as psum:

            # reassemble the full x table from the 8 per-core row shards
            # (collectives cannot read IO tensors; bounce through internal)
            nc.scalar.dma_start(out=xstage[:], in_=xtsh[:])
            nc.gpsimd.collective_compute(
                "AllGather", mybir.AluOpType.bypass,
                replica_groups=[core_ids],
                ins=[xstage[:]], outs=[xfull[:]])

            # index tables: shipped as one 16-partition block, replicated
            # on-device into the 8 partition groups dma_gather expects
            aidx_t = pool.tile([128, NIDX // 16], mybir.dt.int16)
            xidx_t = pool.tile([128, NIDX // 16], mybir.dt.int16)
            for g in range(8):
                nc.sync.dma_start(out=aidx_t[16 * g:16 * (g + 1), :], in_=aidx_in[:])
                nc.sync.dma_start(out=xidx_t[16 * g:16 * (g + 1), :], in_=xidx_in[:])
            wsm_t = pool.tile([128, KTOT], f16)
            nc.sync.dma_start(out=wsm_t[:], in_=w_in[:])
            fshard = pool.tile([128, 4, B], f32)
            nc.sync.dma_start(out=fshard[:], in_=fshard_in[:].rearrange("p (j b) -> p j b", j=4))

            ones128 = pool.tile([128, 1], f32)
            nc.vector.memset(ones128[:], 1.0)

            # alpha shard [p, tile, utt]
            acur = pool.tile([128, 4, B], f32)
            init_view = bass.AP(init64_in.tensor, 0,
                                [(64, 128), (128 * 64, 4), (1, B)])
            nc.sync.dma_start(out=acur[:], in_=init_view)
            # shard64 internal := initial shard
            nc.scalar.dma_start(out=shard64[:], in_=init64_in[:])

            ga = pool.tile([128, KTOT, 64], f32)
            gx = pool.tile([128, KTOT, XCH * B], i8)
            gx16 = pool.tile([128, KTOT, XCH * B], f16)
            cbI = pool.tile([128, 128], f32)
            scap = pool.tile([128, 4, B], f32)
            acc = pool.tile([128, 4, B], f32)
            nc.vector.memset(acc[:], 0.0)

            for t in range(n_steps):
                T_dst = TT[t % 2]
                q = t % XCH
                ch = t // XCH

                # 1. exchange shards -> full table for this step
                if ablate == "noag":
                    T_dst = TT[0]
                else:
                    nc.gpsimd.collective_compute(
                        "AllGather", mybir.AluOpType.bypass,
                        replica_groups=[core_ids],
                        ins=[shard64[:]], outs=[T_dst[:]])

                # 2. merged gathers, split only at the firmware 4096 cap;
                #    per chunk: E' = w * exp(s*q) for all 8 steps at once
                if q == 0 and ablate != "noxg":
                    for o in range(0, NIDX, GCAP):
                        n = min(GCAP, NIDX - o)
                        nc.gpsimd.dma_gather(
                            gx[:, o // 128:(o + n) // 128, :],
                            xfull[ch * D:(ch + 1) * D, :],
                            xidx_t[:, o // 16:(o + n) // 16], n, n,
                            XCH * B, single_packet=False)
                    nc.scalar.activation(
                        out=gx16[:], in_=gx[:],
                        func=mybir.ActivationFunctionType.Exp, scale=XQS)
                    wb = wsm_t[:].unsqueeze(2).unsqueeze(3) \
                        .to_broadcast([128, KTOT, XCH, B])
                    nc.vector.tensor_tensor(
                        out=gx16[:].rearrange("p k (s b) -> p k s b", s=XCH),
                        in0=gx16[:].rearrange("p k (s b) -> p k s b", s=XCH),
                        in1=wb, op=mybir.AluOpType.mult)
                if ablate != "noga":
                    for o in range(0, NIDX, GCAP):
                        n = min(GCAP, NIDX - o)
                        nc.gpsimd.dma_gather(
                            ga[:, o // 128:(o + n) // 128, :], T_dst[:],
                            aidx_t[:, o // 16:(o + n) // 16], n, n, 64,
                            single_packet=False)

                # 3. z = a_src * E' over the whole grid
                if ablate == "nodve":
                    nc.vector.memset(acur[:], 1.0)
                else:
                    gav = ga[:, :, 0:B]
                    nc.vector.tensor_tensor(
                        out=gav, in0=gav,
                        in1=gx16[:, :, q * B:(q + 1) * B],
                        op=mybir.AluOpType.mult)
                    # 4. per-tile reduce over slots
                    for j in range(4):
                        nc.vector.tensor_reduce(
                            out=acur[:, j, :],
                            in_=ga[:, offs[j]:offs[j + 1], 0:B].transpose([0, 2, 1]),
                            axis=mybir.AxisListType.X,
                            op=mybir.AluOpType.add)

                # 5. capture-at-end: utts whose length is t+1 contribute
                #    alpha_{t+1} * exp(final_lp) into the accumulator now;
                #    no per-step freezing needed (later garbage never read).
                if t in end_row:
                    irow = bass.AP(iend_in.tensor, end_row[t] * 128,
                                   [(0, 128), (1, 128)])
                    nc.sync.dma_start(out=cbI[:], in_=irow)
                    nc.vector.tensor_tensor(out=scap[:], in0=acur[:], in1=fshard[:],
                                            op=mybir.AluOpType.mult)
                    ci = cbI[:, 0:B].unsqueeze(1).to_broadcast([128, 4, B])
                    nc.vector.tensor_tensor(out=scap[:], in0=scap[:], in1=ci,
                                            op=mybir.AluOpType.mult)
                    nc.vector.tensor_tensor(out=acc[:], in0=acc[:], in1=scap[:],
                                            op=mybir.AluOpType.add)

                # 6. write shard for next exchange
                sh_view = bass.AP(shard64.tensor, 0, [(64, 128), (128 * 64, 4), (1, B)])
                nc.sync.dma_start(out=sh_view, in_=acur[:])

            # ---- final partials (from the end-capture accumulator) ----
            pd = psum.tile([1, 4 * B], f32, space="PSUM")
            nc.tensor.matmul(out=pd[:], lhsT=ones128[:],
                             rhs=acc[:], start=True, stop=True)
            den_part = pool.tile([1, B], f32)
            nc.vector.tensor_reduce(
                out=den_part[:],
                in_=pd[:].rearrange("o (j b) -> o j b", j=4).transpose([0, 2, 1]),
                axis=mybir.AxisListType.X, op=mybir.AluOpType.add)

            nc.sync.dma_start(out=out_t[0:1, :], in_=den_part[:])

    nc.compile()
    return nc


_CACHE = {}


def _get_program(Kmax, n_steps, ends, ablate=""):
    key = (tuple(Kmax), n_steps, tuple(ends), ablate)
    if key not in _CACHE:
        _CACHE[key] = _build(Kmax, n_steps, ends, ablate)
    return _CACHE[key]


LAST_EXEC_NS = None
LAST_RUN_S = None


def kernel(x, x_lengths, den_src, den_dst, den_pdf, den_logw, den_init, den_final,
           num_src, num_dst, num_pdf, num_logw, num_init, num_final,
           n_steps=T, _want_results=False, _trace=False, _ablate=""):
    global LAST_EXEC_NS, LAST_RUN_S
    import time as _time
    from concourse.bass_utils import run_bass_kernel_spmd

    x = np.asarray(x, np.float32)
    x_lengths_np = np.asarray(x_lengths)
    den_logw = np.asarray(den_logw, np.float64)

    # static rescale: fold expected per-step growth into the edge weights
    mexp = float(np.mean(np.exp(x[:, ::7, ::5], dtype=np.float64)))
    c_den = float(np.log(np.exp(den_logw).sum() / S_DEN * mexp))
    den_logw_adj = (den_logw - c_den).astype(np.float32)

    per_core, Kmax, rowof_den = _preprocess(
        np.asarray(den_src), np.asarray(den_dst), np.asarray(den_pdf),
        den_logw_adj)
    KTOT = sum(Kmax)

    A0 = np.zeros((NROWS,), np.float32)
    A0[rowof_den] = np.exp(np.asarray(den_init)).astype(np.float32)
    F0 = np.zeros((NROWS,), np.float32)
    F0[rowof_den] = np.exp(np.asarray(den_final)).astype(np.float32)

    # x -> time-chunked transpose: row (ch*D + p) = x[:, 8ch:8ch+8, p] flat.
    # Shipped int8 (linear quant, scale XQS), row-sharded over cores; the
    # device AllGathers the full table and dequantizes inside the exp.
    xq = np.clip(np.round(x * (1.0 / XQS)), -127, 127).astype(np.int8)
    TP = NCHUNK * XCH                            # 504 (padded)
    xqt = np.zeros((TP, D, B), np.int8)
    xqt[:T] = xq.transpose(1, 2, 0)              # [T, D, B]
    xt4 = np.ascontiguousarray(
        xqt.reshape(NCHUNK, XCH, D, B)
           .transpose(0, 2, 1, 3)                # [NCHUNK, D, XCH, B]
           .reshape(NCHUNK * D, XCH * B))
    XSH = NCHUNK * D // NCORES

    # end-capture indicator table: one row per distinct utterance-end step
    steps_u = np.minimum(x_lengths_np, n_steps).astype(np.int64)
    ends = sorted(set((steps_u - 1).tolist()))
    iend = np.zeros((max(len(ends), 1), 128), np.float32)
    for i, te in enumerate(ends):
        iend[i, 0:B] = (steps_u - 1 == te).astype(np.float32)

    in_maps = []
    for c in range(NCORES):
        pc = per_core[c]
        aflat = np.concatenate([pc["aidx"][j].T.reshape(-1) for j in range(4)])
        xflat = np.concatenate([pc["xidx"][j].T.reshape(-1) for j in range(4)])
        # index order: i = (off_j + k)*128 + p  -> per tile k-major, partition
        # fastest; aidx[j].T is [K, 128] -> reshape(-1) gives exactly that.
        init64 = np.zeros((SHARD, 64), np.float32)
        init64[:, 0:B] = A0[c * SHARD:(c + 1) * SHARD, None]
        fshard = np.zeros((128, 4 * B), np.float32)
        for j in range(4):
            fshard[:, j * B:(j + 1) * B] = \
                F0[c * SHARD + j * 128:c * SHARD + (j + 1) * 128, None]
        wsm = np.zeros((128, KTOT), np.float16)
        col = 0
        for j in range(4):
            wsm[:, col:col + Kmax[j]] = pc["w"][j]
            col += Kmax[j]
        in_maps.append({
            "xtsh": xt4[c * XSH:(c + 1) * XSH],
            "aidx": np.ascontiguousarray(
                aflat.astype(np.int16).reshape(-1, 16).T),
            "xidx": np.ascontiguousarray(
                xflat.astype(np.int16).reshape(-1, 16).T),
            "wsm": wsm,
            "fshard": fshard,
            "init64": init64,
            "iend": iend,
        })

    # exact numerator log-likelihoods on host (tiny graphs)
    num_ll = _num_ll_host(x, x_lengths_np, np.asarray(num_src),
                          np.asarray(num_dst), np.asarray(num_pdf),
                          np.asarray(num_logw), np.asarray(num_init),
                          np.asarray(num_final), n_steps)

    nc = _get_program(Kmax, n_steps, ends, _ablate)
    _t0 = _time.time()
    try:
        res = run_bass_kernel_spmd(nc, in_maps, core_ids=list(range(NCORES)),
                                   trace=_trace)
    except ModuleNotFoundError:
        # NTFF profiling hooks unavailable in this environment
        res = run_bass_kernel_spmd(nc, in_maps, core_ids=list(range(NCORES)))
    LAST_RUN_S = _time.time() - _t0
    if _trace and res.exec_time_ns:
        LAST_EXEC_NS = res.exec_time_ns
    outs = [res.results[c]["out"] for c in range(NCORES)]
    if _want_results:
        return outs, res

    den_tot = np.sum([o[0] for o in outs], axis=0)
    steps_f = steps_u.astype(np.float64)
    den_ll = np.log(np.maximum(den_tot, 1e-300)) + c_den * steps_f
    objf = -(num_ll.sum() - den_ll.sum()) / x_lengths_np.sum()
    return np.float32(objf)


# revision 44
# speedup vs baseline: 1.1100x; 1.1100x over previous
"""Trainium2 Bass kernel for nn_ChainLoss (LF-MMI style chain loss).

Split by graph size:
  - The 32 per-utterance numerator graphs are tiny (200 states, 600 edges);
    their forward recursions run EXACTLY on the host (vectorized float64
    numpy with per-step renormalisation, ~0.2s) while the device handles the
    heavy shared denominator graph (4000 states, 120k edges, 500 steps,
    batch 32).
  - Denominator on device, in exp-domain with STATIC rescaling: the expected
    per-step growth (from data statistics) is folded into the edge weights
    (w' = w * e^-c) so alpha stays within f32 range for the whole recursion
    (measured drift ~ +8 nats); the host adds c*len_u back at the end.

Device layout: state table A[4096 rows x 32 utts] (f32, stored 64-wide for
256B gather alignment). The 8 cores shard states: core c owns rows
512c..512c+511 (global in-degree round-robin relabel) and all in-edges
targeting them, pre-sorted into a padded grid of 4 partition-tiles.

The per-instruction dispatch overhead dominates on this target, so the step
loop is built from as few instructions as possible:
  AllGather shards -> table T; A[src] rows gathered in ceil(NIDX/4096)
  dma_gathers (firmware cap); x rows gathered once per 8-step chunk from an
  int8 table (256B descriptors); one Exp activation (int8 in, dequant via
  activation scale); two tensor_tensor mults over the whole [128, KTOT, B]
  grid; 4 per-tile reduces; shard writeback. Per-utterance lengths are
  handled by capture-at-end: at the <=32 distinct utterance-ending steps,
  alpha*exp(final_lp) is accumulated (masked by a DMA-broadcast indicator
  row); no per-step freezing is needed since later alpha values for ended
  utterances are never read.

Input staging (the dominant cost over the axon tunnel) is minimized: the
196MB x table is shipped int8 (linear quant, scale 6/127) and row-sharded
across the 8 cores (6MB each), then AllGathered on-device; index tables are
shipped as one 16-partition block and replicated on device; the w grid is
shipped as one weight per edge slot.
"""
import numpy as np

NCORES = 8
B = 32
T = 500
D = 3072
S_DEN = 4000
S_NUM = 200
SHARD = 512
NROWS = SHARD * NCORES      # 4096
XCH = 8                     # time steps per X-gather descriptor/chunk
NCHUNK = -(-T // XCH)       # 63 (time padded to 504)
XQS = 6.0 / 127.0           # int8 x quantization scale (randn tail-safe)


# ---------------------------------------------------------------- host prep
def _preprocess(den_src, den_dst, den_pdf, den_logw):
    indeg = np.bincount(den_dst, minlength=S_DEN)
    rank_of_state = np.empty(S_DEN, np.int64)
    rank_of_state[np.argsort(-indeg, kind="stable")] = np.arange(S_DEN)
    core_of = rank_of_state % NCORES
    rowin = rank_of_state // NCORES
    rowof_den = core_of * SHARD + rowin

    E = len(den_dst)
    core_e = core_of[den_dst]
    ri_e = rowin[den_dst]
    grp = core_e * SHARD + ri_e
    order = np.argsort(grp, kind="stable")
    grp_s = grp[order]
    first = np.r_[True, grp_s[1:] != grp_s[:-1]]
    start_pos = np.where(first, np.arange(E), 0)
    k_within = np.arange(E) - np.maximum.accumulate(start_pos)
    e_src = rowof_den[den_src[order]]
    e_pdf = den_pdf[order]
    e_w = np.exp(den_logw[order]).astype(np.float32)
    tile_s = ri_e[order] // 128
    part_s = ri_e[order] % 128
    core_s = core_e[order]

    per_core = [dict(aidx=[None] * 4, xidx=[None] * 4, w=[None] * 4)
                for _ in range(NCORES)]
    Kmax = [0] * 4
    raw = {}
    for c in range(NCORES):
        for j in range(4):
            sel = (core_s == c) & (tile_s == j)
            K = int(k_within[sel].max()) + 1 if sel.any() else 1
            Kmax[j] = max(Kmax[j], K)
            raw[(c, j)] = sel


    for c in range(NCORES):
        for j in range(4):
            K = Kmax[j]
            sel = raw[(c, j)]
            ai = np.zeros((128, K), np.int32)
            xi = np.zeros((128, K), np.int32)
            wt = np.zeros((128, K), np.float32)
            p, k = part_s[sel], k_within[sel]
            ai[p, k] = e_src[sel]
            xi[p, k] = e_pdf[sel]
            wt[p, k] = e_w[sel]
            pc = per_core[c]
            pc["aidx"][j] = ai; pc["xidx"][j] = xi; pc["w"][j] = wt

    return per_core, Kmax, rowof_den


def _num_ll_host(x, x_lengths, num_src, num_dst, num_pdf, num_logw,
                 num_init, num_final, n_steps):
    """Exact numerator forward recursions, vectorized over utterances.
    x_lengths is sorted descending, so the active set is always a prefix."""
    steps_u = np.minimum(x_lengths, n_steps).astype(np.int64)
    w = np.exp(num_logw.astype(np.float64))            # [B, E]
    a = np.exp(num_init.astype(np.float64))            # [B, S]
    logs = np.zeros(B)
    ui = np.arange(B)[:, None]
    flat_dst = ui * S_NUM + num_dst                    # [B, E]
    for t in range(int(steps_u.max())):
        k = int((steps_u > t).sum())                   # active prefix
        xp = x[np.arange(k)[:, None], t, num_pdf[:k]].astype(np.float64)
        s = a[np.arange(k)[:, None], num_src[:k]] * w[:k] * np.exp(xp)
        anew = np.bincount(flat_dst[:k].ravel(), weights=s.ravel(),
                           minlength=k * S_NUM)[:k * S_NUM].reshape(k, S_NUM)
        g = anew.sum(axis=1)
        logs[:k] += np.log(g)
        a[:k] = anew / g[:, None]
    fin = (a * np.exp(num_final.astype(np.float64))).sum(axis=1)
    return np.log(fin) + logs


# ------------------------------------------------------------- bass program
def _build(Kmax, n_steps, ends, ablate=""):
    import concourse.bass as bass
    import concourse.tile as tile
    from concourse import bacc, mybir

    f32 = mybir.dt.float32
    f16 = mybir.dt.float16
    i8 = mybir.dt.int8
    KTOT = sum(Kmax)
    NIDX = 128 * KTOT
    offs = np.cumsum([0] + Kmax).tolist()
    NEND = max(len(ends), 1)
    end_row = {t: i for i, t in enumerate(ends)}
    GCAP = 4096                 # firmware cap on indices per dma_gather

    nc = bacc.Bacc("TRN2", target_bir_lowering=False, debug=False,
                   num_devices=NCORES, num_swdge_queues=4)
    core_ids = list(range(NCORES))

    XSH = NCHUNK * D // NCORES
    xtsh = nc.dram_tensor("xtsh", [XSH, XCH * B], i8, kind="ExternalInput").ap()
    aidx_in = nc.dram_tensor("aidx", [16, NIDX // 16], mybir.dt.int16, kind="ExternalInput").ap()
    xidx_in = nc.dram_tensor("xidx", [16, NIDX // 16], mybir.dt.int16, kind="ExternalInput").ap()
    w_in = nc.dram_tensor("wsm", [128, KTOT], f16, kind="ExternalInput").ap()
    fshard_in = nc.dram_tensor("fshard", [128, 4 * B], f32, kind="ExternalInput").ap()
    init64_in = nc.dram_tensor("init64", [SHARD, 64], f32, kind="ExternalInput").ap()
    iend_in = nc.dram_tensor("iend", [NEND, 128], f32, kind="ExternalInput").ap()
    out_t = nc.dram_tensor("out", [1, B], f32, kind="ExternalOutput").ap()

    shard64 = nc.dram_tensor("shard64", [SHARD, 64], f32).ap()
    TT = [nc.dram_tensor(f"table{i}", [NROWS, 64], f32, addr_space="Shared").ap()
          for i in range(2)]
    xstage = nc.dram_tensor("xstage", [XSH, XCH * B], i8).ap()
    xfull = nc.dram_tensor("xfull", [NCHUNK * D, XCH * B], i8,
                           addr_space="Shared").ap()

    with tile.TileContext(nc) as tc:
        with tc.tile_pool(name="main", bufs=1) as pool, \
             tc.tile_pool(name="psum", bufs=1, space="PSUM") as psum:

            # reassemble the full x table from the 8 per-core row shards
            # (collectives cannot read IO tensors; bounce through internal)
            nc.scalar.dma_start(out=xstage[:], in_=xtsh[:])
            nc.gpsimd.collective_compute(
                "AllGather", mybir.AluOpType.bypass,
                replica_groups=[core_ids],
                ins=[xstage[:]], outs=[xfull[:]])

            # index tables: shipped as one 16-partition block, replicated
            # on-device into the 8 partition groups dma_gather expects
            aidx_t = pool.tile([128, NIDX // 16], mybir.dt.int16)
            xidx_t = pool.tile([128, NIDX // 16], mybir.dt.int16)
            for g in range(8):
                nc.sync.dma_start(out=aidx_t[16 * g:16 * (g + 1), :], in_=aidx_in[:])
                nc.sync.dma_start(out=xidx_t[16 * g:16 * (g + 1), :], in_=xidx_in[:])
            wsm_t = pool.tile([128, KTOT], f16)
            nc.sync.dma_start(out=wsm_t[:], in_=w_in[:])
            fshard = pool.tile([128, 4, B], f32)
            nc.sync.dma_start(out=fshard[:], in_=fshard_in[:].rearrange("p (j b) -> p j b", j=4))

            ones128 = pool.tile([128, 1], f32)
            nc.vector.memset(ones128[:], 1.0)

            # alpha shard [p, tile, utt]
            acur = pool.tile([128, 4, B], f32)
            init_view = bass.AP(init64_in.tensor, 0,
                                [(64, 128), (128 * 64, 4), (1, B)])
            nc.sync.dma_start(out=acur[:], in_=init_view)
            # shard64 internal := initial shard
            nc.scalar.dma_start(out=shard64[:], in_=init64_in[:])

            ga = pool.tile([128, KTOT, 64], f32)
            gx = pool.tile([128, KTOT, XCH * B], i8)
            gx16 = pool.tile([128, KTOT, XCH * B], f16)
            cbI = pool.tile([128, 128], f32)
            scap = pool.tile([128, 4, B], f32)
            acc = pool.tile([128, 4, B], f32)
            nc.vector.memset(acc[:], 0.0)

            for t in range(n_steps):
                T_dst = TT[t % 2]
                q = t % XCH
                ch = t // XCH

                # 1. exchange shards -> full table for this step
                if ablate == "noag":
                    T_dst = TT[0]
                else:
                    nc.gpsimd.collective_compute(
                        "AllGather", mybir.AluOpType.bypass,
                        replica_groups=[core_ids],
                        ins=[shard64[:]], outs=[T_dst[:]])

                # 2. merged gathers, split only at the firmware 4096 cap;
                #    per chunk: E' = w * exp(s*q) for all 8 steps at once
                if q == 0 and ablate != "noxg":
                    for o in range(0, NIDX, GCAP):
                        n = min(GCAP, NIDX - o)
                        nc.gpsimd.dma_gather(
                            gx[:, o // 128:(o + n) // 128, :],
                            xfull[ch * D:(ch + 1) * D, :],
                            xidx_t[:, o // 16:(o + n) // 16], n, n,
                            XCH * B, single_packet=False,
                            queue_num=(o // GCAP) % 4)
                    nc.scalar.activation(
                        out=gx16[:], in_=gx[:],
                        func=mybir.ActivationFunctionType.Exp, scale=XQS)
                    wb = wsm_t[:].unsqueeze(2).unsqueeze(3) \
                        .to_broadcast([128, KTOT, XCH, B])
                    nc.vector.tensor_tensor(
                        out=gx16[:].rearrange("p k (s b) -> p k s b", s=XCH),
                        in0=gx16[:].rearrange("p k (s b) -> p k s b", s=XCH),
                        in1=wb, op=mybir.AluOpType.mult)
                if ablate != "noga":
                    for o in range(0, NIDX, GCAP):
                        n = min(GCAP, NIDX - o)
                        nc.gpsimd.dma_gather(
                            ga[:, o // 128:(o + n) // 128, :], T_dst[:],
                            aidx_t[:, o // 16:(o + n) // 16], n, n, 64,
                            single_packet=False,
                            queue_num=(o // GCAP) % 4)

                # 3. z = a_src * E' over the whole grid
                if ablate == "nodve":
                    nc.vector.memset(acur[:], 1.0)
                else:
                    gav = ga[:, :, 0:B]
                    nc.vector.tensor_tensor(
                        out=gav, in0=gav,
                        in1=gx16[:, :, q * B:(q + 1) * B],
                        op=mybir.AluOpType.mult)
                    # 4. per-tile reduce over slots
                    for j in range(4):
                        nc.vector.tensor_reduce(
                            out=acur[:, j, :],
                            in_=ga[:, offs[j]:offs[j + 1], 0:B].transpose([0, 2, 1]),
                            axis=mybir.AxisListType.X,
                            op=mybir.AluOpType.add)

                # 5. capture-at-end: utts whose length is t+1 contribute
                #    alpha_{t+1} * exp(final_lp) into the accumulator now;
                #    no per-step freezing needed (later garbage never read).
                if t in end_row:
                    irow = bass.AP(iend_in.tensor, end_row[t] * 128,
                                   [(0, 128), (1, 128)])
                    nc.sync.dma_start(out=cbI[:], in_=irow)
                    nc.vector.tensor_tensor(out=scap[:], in0=acur[:], in1=fshard[:],
                                            op=mybir.AluOpType.mult)
                    ci = cbI[:, 0:B].unsqueeze(1).to_broadcast([128, 4, B])
                    nc.vector.tensor_tensor(out=scap[:], in0=scap[:], in1=ci,
                                            op=mybir.AluOpType.mult)
                    nc.vector.tensor_tensor(out=acc[:], in0=acc[:], in1=scap[:],
                                            op=mybir.AluOpType.add)

                # 6. write shard for next exchange
                sh_view = bass.AP(shard64.tensor, 0, [(64, 128), (128 * 64, 4), (1, B)])
                nc.sync.dma_start(out=sh_view, in_=acur[:])

            # ---- final partials (from the end-capture accumulator) ----
            pd = psum.tile([1, 4 * B], f32, space="PSUM")
            nc.tensor.matmul(out=pd[:], lhsT=ones128[:],
                             rhs=acc[:], start=True, stop=True)
            den_part = pool.tile([1, B], f32)
            nc.vector.tensor_reduce(
                out=den_part[:],
                in_=pd[:].rearrange("o (j b) -> o j b", j=4).transpose([0, 2, 1]),
                axis=mybir.AxisListType.X, op=mybir.AluOpType.add)

            nc.sync.dma_start(out=out_t[0:1, :], in_=den_part[:])

    nc.compile()
    return nc


_CACHE = {}


def _get_program(Kmax, n_steps, ends, ablate=""):
    key = (tuple(Kmax), n_steps, tuple(ends), ablate)
    if key not in _CACHE:
        _CACHE[key] = _build(Kmax, n_steps, ends, ablate)
    return _CACHE[key]


LAST_EXEC_NS = None
LAST_RUN_S = None


def kernel(x, x_lengths, den_src, den_dst, den_pdf, den_logw, den_init, den_final,
           num_src, num_dst, num_pdf, num_logw, num_init, num_final,
           n_steps=T, _want_results=False, _trace=False, _ablate=""):
    global LAST_EXEC_NS, LAST_RUN_S
    import time as _time
    from concourse.bass_utils import run_bass_kernel_spmd

    x = np.asarray(x, np.float32)
    x_lengths_np = np.asarray(x_lengths)
    den_logw = np.asarray(den_logw, np.float64)

    # static rescale: fold expected per-step growth into the edge weights
    mexp = float(np.mean(np.exp(x[:, ::7, ::5], dtype=np.float64)))
    c_den = float(np.log(np.exp(den_logw).sum() / S_DEN * mexp))
    den_logw_adj = (den_logw - c_den).astype(np.float32)

    per_core, Kmax, rowof_den = _preprocess(
        np.asarray(den_src), np.asarray(den_dst), np.asarray(den_pdf),
        den_logw_adj)
    KTOT = sum(Kmax)

    A0 = np.zeros((NROWS,), np.float32)
    A0[rowof_den] = np.exp(np.asarray(den_init)).astype(np.float32)
    F0 = np.zeros((NROWS,), np.float32)
    F0[rowof_den] = np.exp(np.asarray(den_final)).astype(np.float32)

    # x -> time-chunked transpose: row (ch*D + p) = x[:, 8ch:8ch+8, p] flat.
    # Shipped int8 (linear quant, scale XQS), row-sharded over cores; the
    # device AllGathers the full table and dequantizes inside the exp.
    xq = np.clip(np.round(x * (1.0 / XQS)), -127, 127).astype(np.int8)
    TP = NCHUNK * XCH                            # 504 (padded)
    xqt = np.zeros((TP, D, B), np.int8)
    xqt[:T] = xq.transpose(1, 2, 0)              # [T, D, B]
    xt4 = np.ascontiguousarray(
        xqt.reshape(NCHUNK, XCH, D, B)
           .transpose(0, 2, 1, 3)                # [NCHUNK, D, XCH, B]
           .reshape(NCHUNK * D, XCH * B))
    XSH = NCHUNK * D // NCORES

    # end-capture indicator table: one row per distinct utterance-end step
    steps_u = np.minimum(x_lengths_np, n_steps).astype(np.int64)
    ends = sorted(set((steps_u - 1).tolist()))
    iend = np.zeros((max(len(ends), 1), 128), np.float32)
    for i, te in enumerate(ends):
        iend[i, 0:B] = (steps_u - 1 == te).astype(np.float32)

    in_maps = []
    for c in range(NCORES):
        pc = per_core[c]
        aflat = np.concatenate([pc["aidx"][j].T.reshape(-1) for j in range(4)])
        xflat = np.concatenate([pc["xidx"][j].T.reshape(-1) for j in range(4)])
        # index order: i = (off_j + k)*128 + p  -> per tile k-major, partition
        # fastest; aidx[j].T is [K, 128] -> reshape(-1) gives exactly that.
        init64 = np.zeros((SHARD, 64), np.float32)
        init64[:, 0:B] = A0[c * SHARD:(c + 1) * SHARD, None]
        fshard = np.zeros((128, 4 * B), np.float32)
        for j in range(4):
            fshard[:, j * B:(j + 1) * B] = \
                F0[c * SHARD + j * 128:c * SHARD + (j + 1) * 128, None]
        wsm = np.zeros((128, KTOT), np.float16)
        col = 0
        for j in range(4):
            wsm[:, col:col + Kmax[j]] = pc["w"][j]
            col += Kmax[j]
        in_maps.append({
            "xtsh": xt4[c * XSH:(c + 1) * XSH],
            "aidx": np.ascontiguousarray(
                aflat.astype(np.int16).reshape(-1, 16).T),
            "xidx": np.ascontiguousarray(
                xflat.astype(np.int16).reshape(-1, 16).T),
            "wsm": wsm,
            "fshard": fshard,
            "init64": init64,
            "iend": iend,
        })

    # exact numerator log-likelihoods on host (tiny graphs)
    num_ll = _num_ll_host(x, x_lengths_np, np.asarray(num_src),
                          np.asarray(num_dst), np.asarray(num_pdf),
                          np.asarray(num_logw), np.asarray(num_init),
                          np.asarray(num_final), n_steps)

    nc = _get_program(Kmax, n_steps, ends, _ablate)
    _t0 = _time.time()
    try:
        res = run_bass_kernel_spmd(nc, in_maps, core_ids=list(range(NCORES)),
                                   trace=_trace)
    except ModuleNotFoundError:
        # NTFF profiling hooks unavailable in this environment
        res = run_bass_kernel_spmd(nc, in_maps, core_ids=list(range(NCORES)))
    LAST_RUN_S = _time.time() - _t0
    if _trace and res.exec_time_ns:
        LAST_EXEC_NS = res.exec_time_ns
    outs = [res.results[c]["out"] for c in range(NCORES)]
    if _want_results:
        return outs, res

    den_tot = np.sum([o[0] for o in outs], axis=0)
    steps_f = steps_u.astype(np.float64)
    den_ll = np.log(np.maximum(den_tot, 1e-300)) + c_den * steps_f
    objf = -(num_ll.sum() - den_ll.sum()) / x_lengths_np.sum()
    return np.float32(objf)
